# revision 15
# baseline (speedup 1.0000x reference)
"""BiDAF attention (nn_BertBidafAttention) on 8 TRN2 NeuronCores — v2.

Math (per batch, reference):
    cp = c @ W.T + b            [CL, H]
    s  = cp @ q.T               [CL, QL]
    s1 = softmax_q(s + qmask_bias)      (row softmax)
    s2 = softmax_c(s + cmask_bias)      (col softmax)
    a  = s1 @ q                 [CL, H]
    bv = (s1 @ s2.T) @ c = s1 @ (s2.T @ c)
    x  = [c, a, c*a, c*bv]      [CL, 4H]

v2 design vs v1 (115 us):
  * fp16 everywhere on-chip (PE 1-pass instead of fp32 4-pass; DMA halved).
    fp16's 11-bit mantissa ~= TF32; the harness gate is 2e-2 absmax-relative
    and v1 measured 1.6e-4, so there is a large precision budget.
  * All layout transposes of INPUTS are done host-side (cT, qT are extra
    DRAM params) -> zero PE/DVE transpose cost for c and q, all loads are
    contiguous full-rate DMAs.  Only the small s-path tiles ([64x128]) are
    PE-transposed on-chip.
  * Both bias terms (q-side qb = q.b + qmask, c-side cmask) are folded into
    the logit accumulation as rank-1 matmuls, computed once.  qb rides into
    ps_st; it cancels in the free-axis softmax (s2) and is exactly what s1
    needs after transpose.  Likewise cbias cancels in s1.
  * Softmax normalization for s2 is deferred into qc (one tensor_scalar per
    half instead of a [64,512] mul + separate copy).
  * The device writes only [a, c*a, c*bv] (3H) in fp16; the c passthrough
    chunk is assembled host-side from the original fp32 input (it is an
    identity copy, not compute).  Device output traffic: 4.7 MB/core.
  * One [128, 3H] store per c-chunk (8 big DMAs/core instead of 24).
  * Loads ride the scalar (ACT) HWDGE ring, stores the sync (SP) ring.

Sharding: data-parallel over batch, 2 batches per core, no collectives.
"""

import numpy as np
from contextlib import ExitStack

import concourse.bass as bass
from concourse import bacc
import concourse.mybir as mybir
import concourse.tile as tile
from concourse.masks import make_identity
from concourse.bass_utils import run_bass_kernel_spmd

B, CL, QL, H = 16, 512, 64, 768
NCORES = 8
BPC = B // NCORES  # batches per core
HK = H // 128      # 6 k-tiles over the feature dims
CT = CL // 128     # 4 c-tiles
NH = H // 2        # 384, N per matmul half (PSUM bank limit: 512 fp32)
NEGB = -1000.0     # additive mask bias; exp(NEGB - max) == 0.0

f32 = mybir.dt.float32
f16 = mybir.dt.float16
i32 = mybir.dt.int32
EXP = mybir.ActivationFunctionType.Exp
COPY = mybir.ActivationFunctionType.Copy
AXX = mybir.AxisListType.X


def _build_nc() -> bass.Bass:
    nc = bacc.Bacc()
    cD = nc.declare_dram_parameter("c", [BPC, CL, H], f16, isOutput=False)
    cTD = nc.declare_dram_parameter("cT", [BPC, H, CL], f16, isOutput=False)
    cTlD = nc.declare_dram_parameter("cT_lo", [BPC, H, CL], f16,
                                     isOutput=False)
    qD = nc.declare_dram_parameter("q", [QL, BPC, H], f16, isOutput=False)
    qTD = nc.declare_dram_parameter("qT", [128, HK, BPC, QL], f16,
                                    isOutput=False)
    WD = nc.declare_dram_parameter("W", [H, H], f16, isOutput=False)
    bD = nc.declare_dram_parameter("b", [H], f16, isOutput=False)
    cmD = nc.declare_dram_parameter("c_mask", [BPC, CL], i32, isOutput=False)
    qmD = nc.declare_dram_parameter("q_mask", [BPC, QL], i32, isOutput=False)
    outD = nc.declare_dram_parameter("out", [BPC, CL, 3 * H], f16,
                                     isOutput=True)

    with tile.TileContext(nc) as tc, ExitStack() as ctx:
        const = ctx.enter_context(tc.tile_pool(name="const", bufs=1))
        wpool = ctx.enter_context(tc.tile_pool(name="wpool", bufs=1))
        perb = ctx.enter_context(tc.tile_pool(name="perb", bufs=2))
        small = ctx.enter_context(tc.tile_pool(name="small", bufs=2))
        outp = ctx.enter_context(tc.tile_pool(name="outp", bufs=3))
        ptp = ctx.enter_context(tc.tile_pool(name="ptp", bufs=2, space="PSUM"))
        pacc = ctx.enter_context(tc.tile_pool(name="pacc", bufs=4,
                                              space="PSUM"))
        pst = ctx.enter_context(tc.tile_pool(name="pst", bufs=2, space="PSUM"))

        ident = const.tile([128, 128], f16)
        make_identity(nc, ident)
        ident32 = const.tile([128, 128], f32)
        make_identity(nc, ident32)
        ones = const.tile([1, CL], f16)
        nc.vector.memset(ones, 1.0)
        # pre-warm the ACT exp table during the DMA head phase
        warm = const.tile([1, 1], f32)
        nc.vector.memset(warm, 0.0)
        warm2 = const.tile([1, 1], f32)
        nc.scalar.activation(warm2, warm, EXP)

        # --- input loads, split across the three DMA writers so they run
        # in parallel: scalar/ACT HWDGE ring (weights + cT), sync/SP ring
        # (c natural + all output stores), gpsimd SWDGE (masks + cT_lo) ---
        qT_sb = wpool.tile([128, HK, BPC, QL], f16)
        nc.scalar.dma_start(out=qT_sb, in_=qTD[:])
        b_sb = wpool.tile([128, HK], f16)
        nc.scalar.dma_start(out=b_sb, in_=bD[:].rearrange("(k p) -> p k",
                                                          p=128))
        w_sb = wpool.tile([128, HK, H], f16)
        for half in range(2):
            ks = slice(half * 3 * 128, (half + 1) * 3 * 128)
            nc.scalar.dma_start(
                out=w_sb[:, half * 3:(half + 1) * 3, :],
                in_=WD[ks, :].rearrange("(k p) h -> p k h", p=128))
        c_nats, cT_sbs, cTl_sbs = [], [], []
        for bi in range(BPC):
            cT_sb = perb.tile([128, HK, CL], f16, tag="cT")
            nc.scalar.dma_start(
                out=cT_sb, in_=cTD[bi].rearrange("(k p) c -> p k c", p=128))
            cT_sbs.append(cT_sb)
            c_nat = perb.tile([128, CT, H], f16, tag="c_nat")
            nc.sync.dma_start(
                out=c_nat, in_=cD[bi].rearrange("(ci p) h -> p ci h", p=128))
            c_nats.append(c_nat)
        q_sb = wpool.tile([QL, BPC, H], f16)
        nc.scalar.dma_start(out=q_sb, in_=qD[:])

        # --- mask bias rows (int32 -> fp32 cast during SWDGE DMA) ---
        cmf = small.tile([1, BPC, CL], f32, tag="cmf", bufs=1)
        nc.gpsimd.dma_start(out=cmf[:1].rearrange("o b l -> o (b l)"),
                            in_=cmD[:].rearrange("b (o l) -> o (b l)", o=1))
        qmf = small.tile([1, BPC * QL], f32, tag="qmf", bufs=1)
        nc.gpsimd.dma_start(out=qmf,
                            in_=qmD[:].rearrange("b (o l) -> o (b l)", o=1))
        for bi in range(BPC):
            cTl_sb = perb.tile([128, HK, CL], f16, tag="cT_lo")
            nc.gpsimd.dma_start(
                out=cTl_sb, in_=cTlD[bi].rearrange("(k p) c -> p k c", p=128))
            cTl_sbs.append(cTl_sb)
        # bias = (mask - 1) * |NEGB|  ->  0 where mask==1, NEGB where mask==0
        cbias = small.tile([1, BPC, CL], f16, tag="cbias", bufs=1)
        nc.scalar.activation(cbias, cmf, COPY, bias=NEGB, scale=-NEGB)
        qmbias = small.tile([1, BPC * QL], f32, tag="qmbias", bufs=1)
        nc.scalar.activation(qmbias, qmf, COPY, bias=NEGB, scale=-NEGB)

        # --- qb[1, (b q)] = b^T qT, then + qmask bias (free-axis layout) ---
        ps_qb = pacc.tile([1, BPC * QL], f32, tag="acc")
        for k in range(HK):
            nc.tensor.matmul(ps_qb, b_sb[:, k:k + 1],
                             qT_sb[:, k].rearrange("p b q -> p (b q)"),
                             start=(k == 0), stop=(k == HK - 1))
        qrow = small.tile([1, BPC, QL], f16, tag="qrow", bufs=1)
        nc.vector.tensor_add(qrow[:1].rearrange("o b q -> o (b q)"), ps_qb,
                             qmbias)

        # --- qwT[h, (b q)] = sum_d W[d,h] qT[d, (b q)]; k-outer, 2 waves.
        # Stored as an f16 hi/lo pair so the logit matmul can compensate
        # the fp16 operand rounding (lo = psum - hi). ---
        qwt = wpool.tile([128, HK, BPC * QL], f16)
        qwl = wpool.tile([128, HK, BPC * QL], f16)
        for wave in range(2):
            hms = range(wave * 3, wave * 3 + 3)
            ps_w = {hm: pacc.tile([128, BPC * QL], f32, tag="acc",
                                  name=f"ps_w{hm}") for hm in hms}
            for k in range(HK):
                rhs = qT_sb[:, k].rearrange("p b q -> p (b q)")
                for hm in hms:
                    nc.tensor.matmul(ps_w[hm],
                                     w_sb[:, k, hm * 128:(hm + 1) * 128],
                                     rhs, start=(k == 0), stop=(k == HK - 1))
            for hm in hms:
                nc.scalar.copy(out=qwt[:, hm, :], in_=ps_w[hm])
                nc.vector.tensor_sub(qwl[:, hm, :], ps_w[hm], qwt[:, hm, :])

        # ===== phased, batch-interleaved emission: the PE queue is strict
        # FIFO, so each batch's dependent stage is emitted a full phase
        # after its producers — the other batch's PE work fills the gap. ==

        # --- phase A: logits sT[q, c] + cmask bias + (qb + qmask) bias.
        # Compensated fp16: hi*hi + hi*lo + lo*hi cross terms. ---
        sTbs, e2Ts, r2s = [], [], []
        for bi in range(BPC):
            cT_sb = cT_sbs[bi]
            cTl_sb = cTl_sbs[bi]
            ps_st = pst.tile([QL, CL], f32, tag="ps_st")
            nc.tensor.matmul(ps_st, ones[:1, :QL], cbias[:1, bi],
                             start=True, stop=False)
            nc.tensor.matmul(ps_st, qrow[:1, bi], ones[:1, :CL],
                             start=False, stop=False)
            bq = slice(bi * QL, (bi + 1) * QL)
            for k in range(HK):
                nc.tensor.matmul(ps_st, qwt[:, k, bq], cT_sb[:, k],
                                 start=False, stop=False)
                nc.tensor.matmul(ps_st, qwt[:, k, bq], cTl_sb[:, k],
                                 start=False, stop=False)
            for k in range(HK):
                nc.tensor.matmul(ps_st, qwl[:, k, bq], cT_sb[:, k],
                                 start=False, stop=(k == HK - 1))
            # consumers of ps_st run on ACT/DVE during the other batch's MMs
            sTb = small.tile([QL, CL], f32, tag="sTb", name=f"sTb{bi}")
            nc.scalar.copy(out=sTb, in_=ps_st)
            sTbs.append(sTb)
            nmax2 = small.tile([QL, 1], f32, tag="nmax2")
            nc.vector.reduce_max(nmax2, ps_st, axis=AXX, negate=True)
            e2T = small.tile([QL, CL], f16, tag="e2T", name=f"e2T{bi}")
            sum2 = small.tile([QL, 1], f32, tag="sum2")
            nc.scalar.activation(e2T, ps_st, EXP, bias=nmax2, scale=1.0,
                                 accum_out=sum2)
            e2Ts.append(e2T)
            r2 = small.tile([QL, 1], f32, tag="r2", name=f"r2{bi}")
            nc.vector.reciprocal(r2, sum2)
            r2s.append(r2)

        # --- phase B: PE transposes of e2T (s2 path) and sTb (s1 path);
        # softmax(q) chains run on ACT/DVE behind the PE stream ---
        s2es, s1s, s1Ts = [], [], []
        for bi in range(BPC):
            s2e = small.tile([128, CT, QL], f16, tag="s2e", name=f"s2e{bi}")
            for ci in range(CT):
                tp = ptp.tile([128, QL], f16, tag="tp")
                nc.tensor.transpose(tp, e2Ts[bi][:, ci * 128:(ci + 1) * 128],
                                    ident[:QL, :QL])
                nc.vector.tensor_copy(out=s2e[:, ci, :], in_=tp)
            s2es.append(s2e)
            s1b = []
            for ci in range(CT):
                ps_s = ptp.tile([128, QL], f32, tag="tp")
                nc.tensor.transpose(ps_s,
                                    sTbs[bi][:, ci * 128:(ci + 1) * 128],
                                    ident32[:QL, :QL])
                nmax1 = small.tile([128, 1], f32, tag="nmax1", bufs=4)
                nc.vector.reduce_max(nmax1, ps_s, axis=AXX, negate=True)
                e1 = small.tile([128, QL], f16, tag="e1", bufs=4)
                sum1 = small.tile([128, 1], f32, tag="sum1", bufs=4)
                nc.scalar.activation(e1, ps_s, EXP, bias=nmax1, scale=1.0,
                                     accum_out=sum1)
                r1 = small.tile([128, 1], f32, tag="r1", bufs=4)
                nc.vector.reciprocal(r1, sum1)
                s1 = small.tile([128, QL], f16, tag="s1", bufs=8,
                                name=f"s1_{bi}_{ci}")
                nc.vector.tensor_scalar_mul(s1, e1, r1)
                s1b.append(s1)
            s1s.append(s1b)

        # --- phase C/D/E interleaved per batch: s1 back-transposes, then
        # qc, then a/bv + stores; batch 0's stores start while batch 1 is
        # still in its s1/qc phases, spreading the store DMA window ---
        def emit_s1T(bi):
            s1T = small.tile([QL, CL], f16, tag="s1T", name=f"s1T{bi}")
            for ci in range(CT):
                tp2 = ptp.tile([QL, 128], f16, tag="tp")
                nc.tensor.transpose(tp2, s1s[bi][ci], ident)
                nc.scalar.copy(out=s1T[:, ci * 128:(ci + 1) * 128], in_=tp2)
            return s1T

        def emit_qc(bi):
            qc = perb.tile([QL, H], f16, tag="qc")
            for hf in range(2):
                cols = slice(hf * NH, (hf + 1) * NH)
                ps_qc = pacc.tile([QL, NH], f32, tag="acc")
                for ci in range(CT):
                    nc.tensor.matmul(ps_qc, s2es[bi][:, ci, :],
                                     c_nats[bi][:, ci, cols],
                                     start=(ci == 0), stop=(ci == CT - 1))
                nc.vector.tensor_scalar_mul(qc[:, cols], ps_qc, r2s[bi])
            return qc

        def emit_ab(bi, s1T, qc):
            c_nat = c_nats[bi]
            for ci in range(CT):
                rows = slice(ci * 128, (ci + 1) * 128)
                ob = outp.tile([128, 3, H], f16, tag="ob")
                for hf in range(2):
                    cols = slice(hf * NH, (hf + 1) * NH)
                    ps_a = pacc.tile([128, NH], f32, tag="acc")
                    nc.tensor.matmul(ps_a, s1T[:, rows], q_sb[:, bi, cols],
                                     start=True, stop=True)
                    ps_bv = pacc.tile([128, NH], f32, tag="acc")
                    nc.tensor.matmul(ps_bv, s1T[:, rows], qc[:, cols],
                                     start=True, stop=True)
                    nc.scalar.copy(out=ob[:, 0, cols], in_=ps_a)
                    nc.vector.tensor_mul(ob[:, 2, cols],
                                         c_nat[:, ci, cols], ps_bv)
                nc.gpsimd.tensor_mul(ob[:, 1, :], c_nat[:, ci, :],
                                     ob[:, 0, :])
                nc.sync.dma_start(out=outD[bi, rows, :],
                                  in_=ob[:].rearrange("p t h -> p (t h)"))

        s1T0 = emit_s1T(0)
        qc0 = emit_qc(0)
        s1T1 = emit_s1T(1)
        emit_ab(0, s1T0, qc0)
        qc1 = emit_qc(1)
        emit_ab(1, s1T1, qc1)

    nc.finalize()
    return nc


_NC_CACHE: dict = {}


def _get_nc() -> bass.Bass:
    if "nc" not in _NC_CACHE:
        _NC_CACHE["nc"] = _build_nc()
    return _NC_CACHE["nc"]


def _core_in_map(c16, cT16, cTl16, q16, W16, b16, c_mask, q_mask,
                 core: int) -> dict:
    sl = slice(core * BPC, (core + 1) * BPC)
    qs = q16[sl]  # [BPC, QL, H]
    # qT[p, k, b, q] = q[b, q, k*128+p]
    qT = np.ascontiguousarray(
        qs.transpose(2, 0, 1).reshape(HK, 128, BPC, QL).transpose(1, 0, 2, 3))
    return {
        "c": np.ascontiguousarray(c16[sl]),
        "cT": np.ascontiguousarray(cT16[sl]),
        "cT_lo": np.ascontiguousarray(cTl16[sl]),
        "q": np.ascontiguousarray(qs.transpose(1, 0, 2)),
        "qT": qT,
        "W": W16,
        "b": b16,
        "c_mask": np.ascontiguousarray(np.asarray(c_mask[sl], dtype=np.int32)),
        "q_mask": np.ascontiguousarray(np.asarray(q_mask[sl], dtype=np.int32)),
    }


def _prep_all(c, q, c_mask, q_mask, W, b):
    c32 = np.asarray(c, dtype=np.float32)
    c16 = c32.astype(np.float16)
    cl16 = (c32 - c16.astype(np.float32)).astype(np.float16)
    cT16 = np.ascontiguousarray(c16.transpose(0, 2, 1))
    cTl16 = np.ascontiguousarray(cl16.transpose(0, 2, 1))
    q16 = np.asarray(q, dtype=np.float16)
    W16 = np.ascontiguousarray(np.asarray(W, dtype=np.float16))
    b16 = np.ascontiguousarray(np.asarray(b, dtype=np.float16))
    return [
        _core_in_map(c16, cT16, cTl16, q16, W16, b16, c_mask, q_mask, i)
        for i in range(NCORES)
    ]


def _assemble(per_core_outs, c) -> np.ndarray:
    out = np.empty((B, CL, 4 * H), dtype=np.float32)
    out[:, :, :H] = np.asarray(c, dtype=np.float32)
    rest = np.concatenate([np.asarray(o, dtype=np.float32)
                           for o in per_core_outs], axis=0)
    out[:, :, H:] = rest.reshape(B, CL, 3 * H)
    return out


def kernel(c, q, c_mask, q_mask, W, b, _trace=False):
    nc = _get_nc()
    in_maps = _prep_all(c, q, c_mask, q_mask, W, b)
    res = run_bass_kernel_spmd(nc, in_maps, core_ids=list(range(NCORES)),
                               trace=_trace)
    out = _assemble([res.results[i]["out"] for i in range(NCORES)], c)
    if _trace:
        return out, res
    return out
